# revision 2
# baseline (speedup 1.0000x reference)
"""MAMConv2d Trainium2 kernel (8-core SPMD, out-channel sharded).

y[b,co,r,w] = max_k(patch*w) + min_k(patch*w) + bias[co],
k over (3x3 taps x 128 cin); x [16,128,32,32], weight [128,128,3,3].

Sharding: 128 output channels split across 8 cores (16 each); every core
processes all 16 images. Positions are tiled 120 at a time (4 output
rows x 30 cols) onto SBUF partitions, K=1152 on the free dim. Patches
are built by 9 tap-DMAs from a host-staged pixel-major copy of x.
Per (tile, filter): one DVE multiply then max- and min-reduce.
"""
import numpy as np

B, CIN, H, W = 16, 128, 32, 32
COUT, KH, KW = 128, 3, 3
HO, WO = H - KH + 1, W - KW + 1  # 30, 30
K = KH * KW * CIN  # 1152
NCORES = 8
CO_PER_CORE = COUT // NCORES  # 16
ROW_STARTS = [0, 4, 8, 12, 16, 20, 24, 26]  # 4-row tiles; last overlaps
ROWS_PER_TILE = 4
POS_PER_TILE = ROWS_PER_TILE * WO  # 120

_CACHE = {}


def _install_drain_patch():
    """This container's walrus build accepts at most ONE sem-wait per
    instruction. Two fixes: (a) Tile's exit drain gets its global-clock
    waits fanned out over single-wait nops; (b) a module-wide post-pass
    (split_sem_waits) hoists extra waits off every instruction."""
    import concourse.mybir as mybir
    from concourse import tile
    from concourse.vector_clock import ScopedClock

    if getattr(tile.TileContext, "_mam_drain_patched", False):
        return

    def _patched(self, tick_clock, wait_clock):
        nc = self.nc
        collector = nc.sync.nop(nofuse=True)
        wait_clock.add_sem_waits(
            collector.ins, ScopedClock({None: tick_clock.global_clock})
        )
        waits = (
            list(collector.ins.sync_info.on_wait or [])
            if collector.ins.sync_info
            else []
        )
        collector.ins.sync_info = mybir.SyncInfo(on_wait=waits[:1], on_update=[])
        for w in waits[1:]:
            n = nc.sync.nop(nofuse=True)
            n.ins.sync_info = mybir.SyncInfo(on_wait=[w], on_update=[])
        nc.sync.drain()
        nc.all_engine_barrier()
        assert self.sems is not None
        popped = nc._tile_sem_poison_stack.pop()
        assert popped is self._sem_poison
        nc.clear_and_free_semaphores(list(self.sems.allocated().values()))
        nc.all_engine_barrier()

    tile.TileContext._drain_and_barrier = _patched
    tile.TileContext._mam_drain_patched = True


def split_sem_waits(nc, limit=1):
    import concourse.mybir as mybir

    n = 0
    for fn in nc.m.functions:
        for bb in fn.blocks:
            cur = bb.instructions
            new = []
            changed = False
            for inst in cur:
                si = inst.sync_info
                if si is not None and si.on_wait and len(si.on_wait) > limit:
                    waits = list(si.on_wait)
                    for w in waits[:-limit]:
                        n += 1
                        new.append(
                            mybir.InstNoOp(
                                name=f"dwsplit{n}-{inst.name}",
                                engine=inst.engine,
                                sync_info=mybir.SyncInfo(on_wait=[w], on_update=[]),
                                bass_nofuse=True,
                            )
                        )
                    inst.sync_info = mybir.SyncInfo(
                        on_wait=waits[-limit:], on_update=list(si.on_update or [])
                    )
                    changed = True
                new.append(inst)
            if changed:
                bb.instructions = new
    return n


def _build_module():
    import concourse.bass as bass
    import concourse.mybir as mybir
    from concourse import tile

    _install_drain_patch()

    F32 = mybir.dt.float32
    AL = mybir.AluOpType
    nc = bass.Bass(trn_type="TRN2")
    # pixel-major x: row (b*1024 + r*32 + w), col = cin
    xt = nc.dram_tensor("xt", [B * H * W, CIN], F32, kind="ExternalInput")
    # this core's filters, k-major: wq[co][k], k = (i*3+j)*128 + c
    wq = nc.dram_tensor("wq", [CO_PER_CORE, K], F32, kind="ExternalInput")
    bq = nc.dram_tensor("bq", [1, CO_PER_CORE], F32, kind="ExternalInput")
    # y[img][pos][co_local], pos = r*30+w
    y = nc.dram_tensor("y", [B, HO * WO, CO_PER_CORE], F32, kind="ExternalOutput")

    xt4 = xt.rearrange("(b r w) c -> b r w c", b=B, r=H, w=W)

    with tile.TileContext(nc) as tc:
        with (
            tc.tile_pool(name="const", bufs=1) as cpool,
            tc.tile_pool(name="work", bufs=3) as wpool,
            tc.tile_pool(name="prod", bufs=2) as ppool,
            tc.tile_pool(name="out", bufs=2) as opool,
        ):
            wb = cpool.tile([128, CO_PER_CORE * K], F32, tag="wb")
            bias = cpool.tile([128, CO_PER_CORE], F32, tag="bias")
            nc.sync.dma_start(
                wb[:, :],
                wq.rearrange("co k -> (co k)")[None, :].to_broadcast(
                    (128, CO_PER_CORE * K)
                ),
            )
            nc.sync.dma_start(
                bias[:, :], bq[0:1, :].to_broadcast((128, CO_PER_CORE))
            )

            for img in range(B):
                for r0 in ROW_STARTS:
                    patch = wpool.tile([128, K], F32, tag="patch")
                    for i in range(KH):
                        for j in range(KW):
                            tap = i * KW + j
                            nc.sync.dma_start(
                                patch[0:POS_PER_TILE, tap * CIN : (tap + 1) * CIN],
                                xt4[
                                    img,
                                    r0 + i : r0 + i + ROWS_PER_TILE,
                                    j : j + WO,
                                    :,
                                ],
                            )
                    accmax = opool.tile([128, CO_PER_CORE], F32, tag="accmax")
                    accmin = opool.tile([128, CO_PER_CORE], F32, tag="accmin")
                    for co in range(CO_PER_CORE):
                        prod = ppool.tile([128, K], F32, tag="prod")
                        nc.vector.tensor_tensor(
                            prod[:, :],
                            patch[:, :],
                            wb[:, co * K : (co + 1) * K],
                            AL.mult,
                        )
                        nc.vector.tensor_reduce(
                            accmax[:, co : co + 1],
                            prod[:, :],
                            mybir.AxisListType.X,
                            AL.max,
                        )
                        nc.vector.tensor_reduce(
                            accmin[:, co : co + 1],
                            prod[:, :],
                            mybir.AxisListType.X,
                            AL.min,
                        )
                    out_t = opool.tile([128, CO_PER_CORE], F32, tag="out_t")
                    nc.vector.tensor_tensor(
                        out_t[:, :], accmax[:, :], accmin[:, :], AL.add
                    )
                    nc.vector.tensor_tensor(
                        out_t[:, :], out_t[:, :], bias[:, :], AL.add
                    )
                    nc.sync.dma_start(
                        y[img, r0 * WO : r0 * WO + POS_PER_TILE, :],
                        out_t[0:POS_PER_TILE, :],
                    )

    split_sem_waits(nc, limit=1)
    return nc


def kernel(x, weight, bias):
    from concourse.bass_utils import run_bass_kernel_spmd

    x = np.asarray(x, dtype=np.float32)
    weight = np.asarray(weight, dtype=np.float32)
    bias = np.asarray(bias, dtype=np.float32)

    if "nc" not in _CACHE:
        _CACHE["nc"] = _build_module()
    nc = _CACHE["nc"]

    xt = np.ascontiguousarray(x.transpose(0, 2, 3, 1).reshape(B * H * W, CIN))
    # wk[co, (i*3+j)*128 + c] = weight[co, c, i, j]
    wk = np.ascontiguousarray(weight.transpose(0, 2, 3, 1).reshape(COUT, K))
    in_maps = []
    for core in range(NCORES):
        sl = slice(core * CO_PER_CORE, (core + 1) * CO_PER_CORE)
        in_maps.append(
            {
                "xt": xt,
                "wq": np.ascontiguousarray(wk[sl]),
                "bq": np.ascontiguousarray(bias[sl]).reshape(1, CO_PER_CORE),
            }
        )

    res = run_bass_kernel_spmd(nc, in_maps, core_ids=list(range(NCORES)))

    parts = [
        res.results[c]["y"].reshape(B, HO, WO, CO_PER_CORE) for c in range(NCORES)
    ]
    full = np.concatenate(parts, axis=-1)  # [B, HO, WO, COUT]
    return np.ascontiguousarray(full.transpose(0, 3, 1, 2))


# revision 7
# speedup vs baseline: 531.4063x; 531.4063x over previous
"""MAMConv2d Trainium2 kernel (8-core SPMD, out-channel sharded).

y[b,co,r,w] = max_k(patch*w) + min_k(patch*w) + bias[co],
k over (3x3 taps x 128 cin); x [16,128,32,32], weight [128,128,3,3].

Sharding: the 128 output channels split across 8 cores (16 each); every
core processes all 16 images.

Per-core pipeline (three engines in a producer chain):
  - TensorE: products via block-diagonal matmuls. x stays resident in
    its native [cin, pixel] layout; for each tap the stationary operand
    is a CONTIGUOUS 128-pixel run (4 rows x 32 pixel-cols; cols 30/31
    are don't-care), and rhs is diag(w[co,:,tap]), so PSUM receives
    exact fp32 products x[c,pix+off]*w[co,c,tap] -- no patch im2col, no
    DMA expansion.
  - ScalarE: copies each 9-tap PSUM slab to SBUF as fp16 (free cast).
  - VectorE: pairwise max/min tree levels at the 2x fp16 rate shrink
    K=1152 -> 72 in-place, then 1x tensor_reduce, add bias, DMA out.

fp16 product rounding keeps |err| ~1e-3 of output scale.

The module carries an `nrep` input looping the whole compute (for
on-device timing); kernel() runs with nrep=1.
"""
import numpy as np

B, CIN, H, W = 16, 128, 32, 32
COUT, KH, KW = 128, 3, 3
HO, WO = H - KH + 1, W - KW + 1  # 30, 30
K = KH * KW * CIN  # 1152
NTAP = KH * KW
NCORES = 8
CO_PER_CORE = COUT // NCORES  # 16
ROW_STARTS = [0, 4, 8, 12, 16, 20, 24, 26]
NPIX = B * H * W  # 16384
XPAD = 256

_CACHE = {}


def _install_drain_patch():
    """This walrus build accepts at most ONE sem-wait per instruction;
    Tile's exit drain carries several. Fan them out over nops."""
    import concourse.mybir as mybir
    from concourse import tile
    from concourse.vector_clock import ScopedClock

    if getattr(tile.TileContext, "_mam_drain_patched", False):
        return

    def _patched(self, tick_clock, wait_clock):
        nc = self.nc
        collector = nc.sync.nop(nofuse=True)
        wait_clock.add_sem_waits(
            collector.ins, ScopedClock({None: tick_clock.global_clock})
        )
        waits = (
            list(collector.ins.sync_info.on_wait or [])
            if collector.ins.sync_info
            else []
        )
        collector.ins.sync_info = mybir.SyncInfo(on_wait=waits[:1], on_update=[])
        for w in waits[1:]:
            n = nc.sync.nop(nofuse=True)
            n.ins.sync_info = mybir.SyncInfo(on_wait=[w], on_update=[])
        nc.sync.drain()
        nc.all_engine_barrier()
        assert self.sems is not None
        popped = nc._tile_sem_poison_stack.pop()
        assert popped is self._sem_poison
        nc.clear_and_free_semaphores(list(self.sems.allocated().values()))
        nc.all_engine_barrier()

    tile.TileContext._drain_and_barrier = _patched
    tile.TileContext._mam_drain_patched = True


def split_sem_waits(nc, limit=1):
    """Module-wide post-pass: hoist extra sem-waits (walrus limit: 1 per
    instruction) onto single-wait NoOps inserted before the instruction."""
    import concourse.mybir as mybir

    n = 0
    for fn in nc.m.functions:
        for bb in fn.blocks:
            cur = bb.instructions
            new = []
            changed = False
            for inst in cur:
                si = inst.sync_info
                if si is not None and si.on_wait and len(si.on_wait) > limit:
                    waits = list(si.on_wait)
                    for w in waits[:-limit]:
                        n += 1
                        new.append(
                            mybir.InstNoOp(
                                name=f"dwsplit{n}-{inst.name}",
                                engine=inst.engine,
                                sync_info=mybir.SyncInfo(on_wait=[w], on_update=[]),
                                bass_nofuse=True,
                            )
                        )
                    inst.sync_info = mybir.SyncInfo(
                        on_wait=waits[-limit:], on_update=list(si.on_update or [])
                    )
                    changed = True
                new.append(inst)
            if changed:
                bb.instructions = new
    return n


def _build_module(stages="all"):
    import concourse.bass as bass
    import concourse.mybir as mybir
    from concourse import tile

    _install_drain_patch()

    F16 = mybir.dt.float16
    F32 = mybir.dt.float32
    AL = mybir.AluOpType
    AX = mybir.AxisListType
    CO = CO_PER_CORE

    nc = bass.Bass(trn_type="TRN2")
    xs = nc.dram_tensor("xs", [128, NPIX + XPAD], F16, kind="ExternalInput")
    rhd = nc.dram_tensor("rhd", [128, CO * NTAP * 128], F16, kind="ExternalInput")
    bq = nc.dram_tensor("bq", [1, CO], F32, kind="ExternalInput")
    nrep = nc.dram_tensor("nrep", [1, 1], mybir.dt.int32, kind="ExternalInput")
    # all 128 raster positions per tile; host trims cols 30/31
    y = nc.dram_tensor("y", [B, len(ROW_STARTS), 128, CO], F32, kind="ExternalOutput")

    with tile.TileContext(nc) as tc:
        with (
            tc.tile_pool(name="const", bufs=1) as cpool,
            tc.tile_pool(name="prodp", bufs=2) as prodp,
            tc.tile_pool(name="treep", bufs=1) as treep,
            tc.tile_pool(name="outp", bufs=2) as outp,
            tc.tile_pool(name="psp", bufs=2, space="PSUM") as psp,
        ):
            x_sb = cpool.tile([128, NPIX + XPAD], F16, tag="x_sb")
            rh_sb = cpool.tile([128, CO, NTAP, 128], F16, tag="rh_sb")
            bias = cpool.tile([128, CO], F32, tag="bias")
            ntile = cpool.tile([1, 1], mybir.dt.int32, tag="ntile")
            nc.sync.dma_start(x_sb[:, :], xs[:, :])
            nc.sync.dma_start(
                rh_sb[:, :, :, :],
                rhd.rearrange("c (co t n) -> c co t n", co=CO, t=NTAP),
            )
            nc.sync.dma_start(bias[:, :], bq[0:1, :].to_broadcast((128, CO)))
            nc.sync.dma_start(ntile[:, :], nrep[:, :])
            n = nc.values_load(
                ntile[0:1, 0:1], min_val=0, max_val=1 << 20,
                skip_runtime_bounds_check=True,
            )
            with tc.For_i(0, n, 1, name="reploop"):
                for img in range(B):
                    for r0 in ROW_STARTS:
                        pixbase = img * H * W + r0 * W
                        prod = prodp.tile([128, CO, K], F16, tag="prod")
                        for g in range(CO):
                            slab = psp.tile([128, NTAP, 128], F32, tag="slab")
                            for i in range(KH):
                                for j in range(KW):
                                    tap = i * KW + j
                                    base = pixbase + i * W + j
                                    nc.tensor.matmul(
                                        slab[:, tap, :],
                                        x_sb[:, base : base + 128],
                                        rh_sb[:, g, tap, :],
                                        start=True,
                                        stop=True,
                                    )
                            if stages != "pe":
                                nc.scalar.copy(prod[:, g, :], slab[:, :, :])
                        if stages in ("pe", "peact"):
                            continue
                        tmin = treep.tile([128, CO, K // 2], F16, tag="tmin")
                        # min tree: l1 -> tmin, then in-place halving
                        nc.vector.tensor_tensor(
                            tmin[:, :, :], prod[:, :, 0:576], prod[:, :, 576:1152], AL.min
                        )
                        # max tree fully in-place in prod
                        nc.vector.tensor_tensor(
                            prod[:, :, 0:576], prod[:, :, 0:576], prod[:, :, 576:1152], AL.max
                        )
                        nc.vector.tensor_tensor(
                            prod[:, :, 0:288], prod[:, :, 0:288], prod[:, :, 288:576], AL.max
                        )
                        nc.vector.tensor_tensor(
                            tmin[:, :, 0:288], tmin[:, :, 0:288], tmin[:, :, 288:576], AL.min
                        )
                        nc.vector.tensor_tensor(
                            prod[:, :, 0:144], prod[:, :, 0:144], prod[:, :, 144:288], AL.max
                        )
                        nc.vector.tensor_tensor(
                            tmin[:, :, 0:144], tmin[:, :, 0:144], tmin[:, :, 144:288], AL.min
                        )
                        nc.vector.tensor_tensor(
                            prod[:, :, 0:72], prod[:, :, 0:72], prod[:, :, 72:144], AL.max
                        )
                        nc.vector.tensor_tensor(
                            tmin[:, :, 0:72], tmin[:, :, 0:72], tmin[:, :, 72:144], AL.min
                        )
                        accmax = outp.tile([128, CO], F32, tag="accmax")
                        accmin = outp.tile([128, CO], F32, tag="accmin")
                        nc.vector.tensor_reduce(
                            accmax[:, :], prod[:, :, 0:72], AX.X, AL.max
                        )
                        nc.vector.tensor_reduce(
                            accmin[:, :], tmin[:, :, 0:72], AX.X, AL.min
                        )
                        out_t = outp.tile([128, CO], F32, tag="out_t")
                        nc.vector.tensor_tensor(
                            out_t[:, :], accmax[:, :], accmin[:, :], AL.add
                        )
                        nc.vector.tensor_tensor(
                            out_t[:, :], out_t[:, :], bias[:, :], AL.add
                        )
                        ti = ROW_STARTS.index(r0)
                        nc.sync.dma_start(y[img, ti, :, :], out_t[:, :])

    split_sem_waits(nc, limit=1)
    return nc


def _in_maps(x, weight, bias, nrep=1):
    # x [B,CIN,H,W] -> [CIN, B*H*W] fp16 (+ pad)
    xs = np.zeros((CIN, NPIX + XPAD), np.float16)
    xs[:, :NPIX] = (
        x.transpose(1, 0, 2, 3).reshape(CIN, NPIX).astype(np.float16)
    )
    narr = np.array([[nrep]], dtype=np.int32)
    maps = []
    ar = np.arange(128)
    for core in range(NCORES):
        sl = slice(core * CO_PER_CORE, (core + 1) * CO_PER_CORE)
        wsh = weight[sl].astype(np.float16)  # [16, 128, 3, 3]
        rh = np.zeros((128, CO_PER_CORE, NTAP, 128), np.float16)
        for co in range(CO_PER_CORE):
            for t in range(NTAP):
                i, j = divmod(t, KW)
                rh[ar, co, t, ar] = wsh[co, :, i, j]
        maps.append(
            {
                "xs": xs,
                "rhd": np.ascontiguousarray(
                    rh.reshape(128, CO_PER_CORE * NTAP * 128)
                ),
                "bq": np.ascontiguousarray(bias[sl])
                .reshape(1, CO_PER_CORE)
                .astype(np.float32),
                "nrep": narr,
            }
        )
    return maps


def _assemble(res):
    parts = []
    for c in range(NCORES):
        yr = res.results[c]["y"].reshape(B, len(ROW_STARTS), 4, 32, CO_PER_CORE)
        out = np.empty((B, HO, WO, CO_PER_CORE), np.float32)
        for ti, r0 in enumerate(ROW_STARTS):
            out[:, r0 : r0 + 4, :, :] = yr[:, ti, :, 0:WO, :]
        parts.append(out)
    full = np.concatenate(parts, axis=-1)  # [B, HO, WO, COUT]
    return np.ascontiguousarray(full.transpose(0, 3, 1, 2))


def kernel(x, weight, bias):
    from concourse.bass_utils import run_bass_kernel_spmd

    x = np.asarray(x, dtype=np.float32)
    weight = np.asarray(weight, dtype=np.float32)
    bias = np.asarray(bias, dtype=np.float32)

    if "nc" not in _CACHE:
        _CACHE["nc"] = _build_module()
    nc = _CACHE["nc"]

    res = run_bass_kernel_spmd(
        nc, _in_maps(x, weight, bias, nrep=1), core_ids=list(range(NCORES))
    )
    return _assemble(res)
